# revision 9
# baseline (speedup 1.0000x reference)
"""Trainium2 Bass kernel for nn_Encoder_67190468378802 (GCN-LSTM encoder), v2.

Architecture (instruction-count-minimal for the axon backend, where every
instruction costs ~40-170us regardless of width):
 - GCN aggregation via dma_gather + dma_scatter_add in duplicate-free
   "rounds": each scatter instruction touches each accumulator slot at most
   once (R=8 replica slots per target cut round count to ~8; chunks of <=64
   tiles). Zero per-edge compute instructions.
 - W2/Wm/Wl commute past the aggregation (linear), applied in wide
   feature-major posts via a handful of 512-wide matmuls.
 - Phase 1 (X@W1) sharded across cores; 3 AllGathers share tables.
 - LSTM: truncated-window recurrence (K=32 warmup), 128 lanes x L=20,
   lane-major (unchanged from v1).
"""
import numpy as np
import ml_dtypes

import concourse.bacc as bacc
import concourse.bass as bass
import concourse.mybir as mybir
import concourse.tile as tile
from concourse.bass_utils import run_bass_kernel_spmd
from concourse.masks import make_identity

F32 = mybir.dt.float32
BF16 = mybir.dt.bfloat16
I16 = mybir.dt.int16
AF = mybir.ActivationFunctionType

N = 20000
NC = 8
SH = N // NC            # 2500
D = 128
G4 = 4 * D
LAT = 64
L = 20
LANES = 128
COVER = LANES * L       # 2560
K_WARM = 10
NT = 20                 # tiles per 2560-range
TP = NT * 128           # 2560 padded targets (own and ext both fit)
REP = 8                 # replica slots per target
TRASH = REP * TP        # trash slot base (rows TRASH..TRASH+127)
ACC_ROWS = REP * TP + 128
CHUNK_TILES = 40        # max gather/scatter tiles per instruction


# ---------------------------------------------------------------- host prep
def _pack_rounds(src, tloc):
    """Duplicate-free round/replica packing.

    Returns (gidx, sidx, chunks): int16 arrays of equal padded length
    (multiple of 128), and a list of per-chunk tile counts. Within any chunk
    all scatter slots are distinct (except the trash slot).
    """
    order = np.lexsort((np.arange(len(tloc)), tloc))
    t_s = tloc[order]
    s_s = src[order]
    # rank of each edge within its target
    uniq, starts = np.unique(t_s, return_index=True)
    rank = np.arange(len(t_s)) - np.repeat(starts, np.diff(np.append(starts, len(t_s))))
    rep = rank % REP
    rnd = rank // REP
    slot = rep * TP + t_s
    nrounds = int(rnd.max()) + 1 if len(rnd) else 0

    gl, sl, chunks = [], [], []
    for r in range(nrounds):
        m = rnd == r
        g = s_s[m]
        s = slot[m]
        n = len(g)
        npad = -(-n // 128) * 128
        gpad = np.zeros(npad, np.int64)
        spad = np.full(npad, TRASH, np.int64)
        gpad[:n] = g
        spad[:n] = s
        ntiles = npad // 128
        o = 0
        while o < ntiles:
            c = min(CHUNK_TILES, ntiles - o)
            chunks.append(c)
            gl.append(gpad[o * 128:(o + c) * 128])
            sl.append(spad[o * 128:(o + c) * 128])
            o += c
    gidx = np.concatenate(gl).astype(np.int16)
    sidx = np.concatenate(sl).astype(np.int16)
    return gidx, sidx, chunks


def _wrap16(idx):
    return np.ascontiguousarray(idx.reshape(-1, 16).T)


def preprocess(edge_index):
    K = K_WARM
    row = np.asarray(edge_index[0], dtype=np.int64)
    col = np.asarray(edge_index[1], dtype=np.int64)
    loop = np.arange(N, dtype=np.int64)
    row = np.concatenate([row, loop])
    col = np.concatenate([col, loop])
    deg = np.bincount(col, minlength=N).astype(np.float64)
    dinv = (1.0 / np.sqrt(deg)).astype(np.float32)

    cores = []
    for c in range(NC):
        start = c * SH
        # own set: targets in [start, start+SH)
        mo = (col >= start) & (col < start + SH)
        g_o, s_o, ch_o = _pack_rounds(row[mo], col[mo] - start)
        # ext set: targets in [start-K, start+SH)
        me = (col >= start - K) & (col < start + SH)
        g_e, s_e, ch_e = _pack_rounds(row[me], col[me] - (start - K))
        cores.append(dict(g_o=g_o, s_o=s_o, ch_o=ch_o,
                          g_e=g_e, s_e=s_e, ch_e=ch_e))

    # shared shapes: pad all cores' index arrays to the max length & chunk
    # schedule must be identical across cores (SPMD single program) -> pad
    # chunk lists with empty?? simplest: use the max schedule; shorter cores
    # pad with full-trash chunks.
    def unify(key_g, key_s, key_ch):
        nch = max(len(co[key_ch]) for co in cores)
        tiles = [max(co[key_ch][i] if i < len(co[key_ch]) else 1 for co in cores)
                 for i in range(nch)]
        for co in cores:
            gl, sl = [], []
            off = 0
            for i, t in enumerate(tiles):
                have = co[key_ch][i] if i < len(co[key_ch]) else 0
                g = np.zeros(t * 128, np.int64)
                s = np.full(t * 128, TRASH, np.int64)
                if have:
                    g[:have * 128] = co[key_g][off:off + have * 128]
                    s[:have * 128] = co[key_s][off:off + have * 128]
                    off += have * 128
                gl.append(g)
                sl.append(s)
            co[key_g + "u"] = _wrap16(np.concatenate(gl).astype(np.int16))
            co[key_s + "u"] = _wrap16(np.concatenate(sl).astype(np.int16))
        return tiles

    tiles_o = unify("g_o", "s_o", "ch_o")
    tiles_e = unify("g_e", "s_e", "ch_e")
    return dict(dinv=dinv, cores=cores, tiles_o=tiles_o, tiles_e=tiles_e, K=K)


# ---------------------------------------------------------------- device
def build_nc(pp, reps=1, stop_after=None):
    K = pp["K"]
    tiles_o, tiles_e = pp["tiles_o"], pp["tiles_e"]
    NIDX_O = sum(tiles_o) * 128
    NIDX_E = sum(tiles_e) * 128
    NXB = -(-(COVER + K) // 128)       # 21 xg row blocks
    NTH = max(NT, NXB)                 # 21
    XGR = NXB * 128
    XGROWS = -(-XGR // L) * L + L * 8

    nc = bacc.Bacc(None, target_bir_lowering=False)

    # ---------------- inputs
    xt = nc.dram_tensor("xt", [D, TP], BF16, kind="ExternalInput")
    w1 = nc.dram_tensor("w1", [D, D], BF16, kind="ExternalInput")
    w2 = nc.dram_tensor("w2", [D, D], BF16, kind="ExternalInput")
    wml = nc.dram_tensor("wml", [D, D], BF16, kind="ExternalInput")
    wiht = nc.dram_tensor("wiht", [D, G4], BF16, kind="ExternalInput")
    whht = nc.dram_tensor("whht", [D, G4], F32, kind="ExternalInput")
    biasg = nc.dram_tensor("biasg", [1, G4], BF16, kind="ExternalInput")
    b2c = nc.dram_tensor("b2c", [D, 1], F32, kind="ExternalInput")
    bmlc = nc.dram_tensor("bmlc", [D, 1], F32, kind="ExternalInput")
    b1r = nc.dram_tensor("b1r", [1, D], BF16, kind="ExternalInput")
    d1all = nc.dram_tensor("d1all", [1, K + TP], F32, kind="ExternalInput")
    d1col = nc.dram_tensor("d1col", [128, NT], F32, kind="ExternalInput")
    d2col = nc.dram_tensor("d2col", [128, NT], F32, kind="ExternalInput")
    dcol20 = nc.dram_tensor("dcol20", [128, L], F32, kind="ExternalInput")
    maskc = nc.dram_tensor("maskc", [128, NXB], F32, kind="ExternalInput")
    gio = nc.dram_tensor("gio", [16, NIDX_O // 16], I16, kind="ExternalInput")
    sio = nc.dram_tensor("sio", [16, NIDX_O // 16], I16, kind="ExternalInput")
    gie = nc.dram_tensor("gie", [16, NIDX_E // 16], I16, kind="ExternalInput")
    sie = nc.dram_tensor("sie", [16, NIDX_E // 16], I16, kind="ExternalInput")
    gior = nc.dram_tensor("gior", [128, NIDX_O // 16], I16)
    sior = nc.dram_tensor("sior", [128, NIDX_O // 16], I16)
    gier = nc.dram_tensor("gier", [128, NIDX_E // 16], I16)
    sier = nc.dram_tensor("sier", [128, NIDX_E // 16], I16)

    # ---------------- outputs
    zmT = nc.dram_tensor("zmT", [LAT, SH], BF16, kind="ExternalOutput")
    zlT = nc.dram_tensor("zlT", [LAT, SH], BF16, kind="ExternalOutput")

    # ---------------- internal DRAM
    t1loc = nc.dram_tensor("t1loc", [TP, D], F32)
    table1 = nc.dram_tensor("table1", [N, D], F32, addr_space="Shared")
    t2loc = nc.dram_tensor("t2loc", [TP, D], F32)
    table2 = nc.dram_tensor("table2", [N, D], F32, addr_space="Shared")
    acc1 = nc.dram_tensor("acc1", [ACC_ROWS, D], F32)
    acc2 = nc.dram_tensor("acc2", [ACC_ROWS, D], F32)
    acc3 = nc.dram_tensor("acc3", [ACC_ROWS, D], F32)
    s2d = nc.dram_tensor("s2d", [TP, D], BF16)
    s3d = nc.dram_tensor("s3d", [TP, D], BF16)
    xg_dram = nc.dram_tensor("xg_dram", [XGROWS, G4], BF16)
    h3tmp = nc.dram_tensor("h3tmp", [COVER, D], F32)
    h3sc = nc.dram_tensor("h3sc", [COVER, D], F32)
    table3 = nc.dram_tensor("table3", [N, D], F32, addr_space="Shared")

    with tile.TileContext(nc) as tc:
        import contextlib
        ctx = contextlib.ExitStack()
        with ctx:
          try:
            const = ctx.enter_context(tc.tile_pool(name="const", bufs=1))
            sb = ctx.enter_context(tc.tile_pool(name="sb", bufs=2))
            ph = ctx.enter_context(tc.tile_pool(name="ph", bufs=1))
            ar = ctx.enter_context(tc.tile_pool(name="ar", bufs=2))
            gat = ctx.enter_context(tc.tile_pool(name="gat", bufs=2))
            ps = ctx.enter_context(tc.tile_pool(name="ps", bufs=2, space="PSUM"))
            psw = ctx.enter_context(tc.tile_pool(name="psw", bufs=2, space="PSUM"))

            # ------------ constants
            gio_t = const.tile([128, NIDX_O // 16], I16)
            sio_t = const.tile([128, NIDX_O // 16], I16)
            gie_t = const.tile([128, NIDX_E // 16], I16)
            sie_t = const.tile([128, NIDX_E // 16], I16)
            for tdst, tsrc, trep in ((gio_t, gio, gior), (sio_t, sio, sior),
                                     (gie_t, gie, gier), (sie_t, sie, sier)):
                for k in range(8):
                    nc.sync.dma_start(trep.ap()[16 * k:16 * (k + 1), :],
                                      tsrc.ap())
                nc.sync.dma_start(tdst[:], trep.ap())
            w1_t = const.tile([128, D], BF16)
            nc.sync.dma_start(w1_t[:], w1[:])
            w2_t = const.tile([128, D], BF16)
            nc.sync.dma_start(w2_t[:], w2[:])
            wml_t = const.tile([128, D], BF16)
            nc.sync.dma_start(wml_t[:], wml[:])
            wih_t = const.tile([128, G4], BF16)
            nc.sync.dma_start(wih_t[:], wiht[:])
            whh_t = const.tile([128, G4], F32)
            nc.sync.dma_start(whh_t[:], whht[:])
            biasg_t = const.tile([1, G4], BF16)
            nc.sync.dma_start(biasg_t[:], biasg[:])
            b2c_t = const.tile([128, 1], F32)
            nc.sync.dma_start(b2c_t[:], b2c[:])
            bmlc_t = const.tile([128, 1], F32)
            nc.sync.dma_start(bmlc_t[:], bmlc[:])
            b1r_t = const.tile([1, D], BF16)
            nc.sync.dma_start(b1r_t[:], b1r[:])
            d1all_t = const.tile([1, K + TP], F32)
            nc.sync.dma_start(d1all_t[:], d1all[:])
            d1col_t = const.tile([128, NT], F32)
            nc.sync.dma_start(d1col_t[:], d1col[:])
            d2col_t = const.tile([128, NT], F32)
            nc.sync.dma_start(d2col_t[:], d2col[:])
            dc20_t = const.tile([128, L], F32)
            nc.sync.dma_start(dc20_t[:], dcol20[:])
            mask_t = const.tile([128, NXB], F32)
            nc.sync.dma_start(mask_t[:], maskc[:])
            ones1_f = const.tile([1, 128], F32)
            nc.vector.memset(ones1_f[:], 1.0)
            ones1_bf = const.tile([1, 128], BF16)
            nc.vector.memset(ones1_bf[:], 1.0)
            ident_f = const.tile([128, 128], F32)
            make_identity(nc, ident_f[:])
            ident_bf = const.tile([128, 128], BF16)
            make_identity(nc, ident_bf[:])
            zeros_t = const.tile([128, 24, 128], F32)
            nc.vector.memset(zeros_t[:], 0.0)

            # d1fm_all [128, K+TP]: dinv over [start-K, start+TP), bcast over
            # partitions (feature-major free-axis scale)
            d1fm = const.tile([128, K + TP], BF16)
            o = 0
            while o < K + TP:
                w_ = min(512, K + TP - o)
                p_ = psw.tile([128, 512], F32, space="PSUM", tag="w")
                nc.tensor.matmul(p_[:, :w_], lhsT=ones1_f[:],
                                 rhs=d1all_t[:, o:o + w_], start=True, stop=True)
                nc.vector.tensor_copy(d1fm[:, o:o + w_], p_[:, :w_])
                o += w_

            # b1w [128, 128] = b1 broadcast over partitions; then
            # b1d [128, NT, 128] = d1(own node) * b1(d)
            b1w_p = psw.tile([128, 512], F32, space="PSUM", tag="w")
            nc.tensor.matmul(b1w_p[:, 0:128], lhsT=ones1_bf[:], rhs=b1r_t[:],
                             start=True, stop=True)
            b1w = const.tile([128, 128], F32)
            nc.vector.tensor_copy(b1w[:], b1w_p[:, 0:128])
            b1d = const.tile([128, NT, 128], BF16)
            for n in range(NT):
                nc.vector.tensor_scalar_mul(b1d[:, n, :], b1w[:],
                                            d1col_t[:, n:n + 1])

            # biasw [128, G4] = biasg broadcast over partitions
            biasw = const.tile([128, G4], F32)
            for o in range(0, G4, 512):
                bp = psw.tile([128, 512], F32, space="PSUM", tag="w")
                nc.tensor.matmul(bp[:], lhsT=ones1_bf[:],
                                 rhs=biasg_t[:, o:o + 512], start=True, stop=True)
                nc.vector.tensor_copy(biasw[:, o:o + 512], bp[:])

            # h2t feature-major [128, NTH*128]; tail zero
            h2t = const.tile([128, NTH * 128], BF16)
            nc.vector.memset(h2t[:, NT * 128:], 0.0)
            h3_sb = const.tile([128, COVER], F32)

            # zero accumulators
            for acc in (acc1, acc2, acc3):
                o = 0
                v = acc.ap().rearrange("(a p) d -> p a d", p=128)
                a_total = ACC_ROWS // 128
                while o < a_total:
                    w_ = min(24, a_total - o)
                    nc.sync.dma_start(v[:, o:o + w_, :], zeros_t[:, 0:w_, :])
                    o += w_

            for _rep in range(reps):
              # ------------ phase 1: t1loc = dinv * (X @ W1) (own shard)
              xt_sb = ph.tile([128, TP], BF16, tag="big3")
              nc.sync.dma_start(xt_sb[:], xt.ap())
              t1fm = ph.tile([128, TP], BF16, tag="cvt")
              for o in range(0, TP, 512):
                  p_ = psw.tile([128, 512], F32, space="PSUM", tag="w")
                  nc.tensor.matmul(p_[:], lhsT=w1_t[:], rhs=xt_sb[:, o:o + 512],
                                   start=True, stop=True)
                  nc.vector.tensor_mul(t1fm[:, o:o + 512], p_[:],
                                       d1fm[:, K + o:K + o + 512])
              nc.sync.dma_start(s2d.ap().rearrange("(n p) d -> p n d", p=128),
                                t1fm[:].rearrange("p (n d) -> p n d", d=128))
              t1nm = ph.tile([128, NT, 128], BF16, tag="fm")
              nc.sync.dma_start(t1nm[:], s2d.ap(), transpose=True)
              t1sb = ph.tile([128, NT, 128], F32, tag="big2")
              nc.vector.tensor_copy(t1sb[:], t1nm[:])
              nc.sync.dma_start(t1loc.ap().rearrange("(n p) d -> p n d", p=128),
                                t1sb[:])
              nc.gpsimd.collective_compute(
                  "AllGather", mybir.AluOpType.bypass,
                  ins=[t1loc.ap()[0:SH, :].opt()],
                  outs=[table1.ap().opt()],
                  replica_groups=[list(range(NC))])
              if stop_after == "p1":
                  raise _StopBuild

              # ------------ conv pass helper: gather+convert+scatter rounds
              def conv_pass(table, acc, gi_t, si_t, tiles_sched, nidx):
                  off = 0
                  for ctiles in tiles_sched:
                      gsz = ctiles * 128
                      gt = gat.tile([128, CHUNK_TILES, D], F32, tag="g")
                      nc.gpsimd.dma_gather(
                          gt[:, 0:ctiles, :], table.ap()[:],
                          gi_t[:, off // 16:(off + gsz) // 16],
                          gsz, gsz, D, single_packet=False)
                      nc.gpsimd.dma_scatter_add(
                          acc.ap()[:], gt[:, 0:ctiles, :],
                          si_t[:, off // 16:(off + gsz) // 16],
                          gsz, gsz, D, single_packet=False)
                      off += gsz

              def load_reduce(acc, out):
                  """accumulating DMA loads: out = sum_r acc[rep r] (CCE add)"""
                  v = acc.ap()[0:REP * TP, :].rearrange(
                      "(r n p) d -> p r n d", p=128, r=REP)
                  nc.sync.dma_start(out[:], v[:, 0, :, :])
                  for r in range(1, REP):
                      nc.gpsimd.dma_start(out[:], v[:, r, :, :],
                                          accum_op=mybir.AluOpType.add)

              # ------------ conv1 -> table2 rows (node-major post)
              conv_pass(table1, acc1, gio_t, sio_t, tiles_o, NIDX_O)
              s1 = ph.tile([128, NT, 128], F32, tag="big1")
              load_reduce(acc1, s1)
              for n in range(NT):
                  nc.vector.tensor_scalar_mul(s1[:, n, :], s1[:, n, :],
                                              d2col_t[:, n:n + 1])
              nc.vector.tensor_add(s1[:], s1[:], b1d[:])
              o1 = ph.tile([128, NT, 128], F32, tag="big2")
              nc.scalar.activation(o1[:], s1[:], AF.Relu)
              nc.sync.dma_start(t2loc.ap().rearrange("(n p) d -> p n d", p=128),
                                o1[:])
              nc.gpsimd.collective_compute(
                  "AllGather", mybir.AluOpType.bypass,
                  ins=[t2loc.ap()[0:SH, :].opt()],
                  outs=[table2.ap().opt()],
                  replica_groups=[list(range(NC))])
              if stop_after == "conv1":
                  raise _StopBuild

              # ------------ conv2 -> h2t feature-major (ext targets)
              conv_pass(table2, acc2, gie_t, sie_t, tiles_e, NIDX_E)
              s2 = ph.tile([128, NT, 128], F32, tag="big1")
              load_reduce(acc2, s2)
              c2 = ph.tile([128, NT, 128], BF16, tag="cvt")
              nc.vector.tensor_copy(c2[:], s2[:])
              nc.sync.dma_start(s2d.ap().rearrange("(n p) d -> p n d", p=128),
                                c2[:])
              fm2 = ph.tile([128, NT, 128], BF16, tag="fm")
              nc.sync.dma_start(fm2[:], s2d.ap(), transpose=True)
              for o in range(0, TP, 512):
                  p_ = psw.tile([128, 512], F32, space="PSUM", tag="w")
                  nc.tensor.matmul(p_[:], lhsT=w2_t[:],
                                   rhs=fm2[:].rearrange("p n d -> p (n d)")[:, o:o + 512],
                                   start=True, stop=True)
                  t_ = sb.tile([128, 512], F32, tag="t2f")
                  nc.vector.tensor_mul(t_[:], p_[:], d1fm[:, o:o + 512])
                  nc.scalar.activation(h2t[:, o:o + 512], t_[:], AF.Relu,
                                       bias=b2c_t[:, 0:1])
              if stop_after == "conv2":
                  raise _StopBuild

              # ------------ xg = H2T.T @ WihT + bias (masked)
              for b in range(NXB):
                  p_ = psw.tile([128, G4], F32, space="PSUM", tag="w")
                  nc.tensor.matmul(p_[:], lhsT=h2t[:, b * 128:(b + 1) * 128],
                                   rhs=wih_t[:], start=True, stop=True)
                  ob = sb.tile([128, G4], F32, tag="xgb")
                  nc.vector.tensor_add(ob[:], p_[:], biasw[:])
                  o_ = sb.tile([128, G4], BF16, tag="xgo")
                  nc.vector.tensor_scalar_mul(o_[:], ob[:], mask_t[:, b:b + 1])
                  nc.sync.dma_start(xg_dram.ap()[b * 128:(b + 1) * 128, :], o_[:])
              if stop_after == "xg":
                  raise _StopBuild

              # ------------ LSTM (lane-major, K warmup)
              c_t = const.tile([128, D], F32)
              nc.vector.memset(c_t[:], 0.0)
              ht_t = const.tile([128, D], F32)
              nc.vector.memset(ht_t[:], 0.0)
              xgv = xg_dram.ap().rearrange("(l r) g -> l r g", r=L)
              for s in range(K + L):
                  q, r = divmod(s, L)
                  xgt_t = sb.tile([128, G4], BF16, tag="xgl")
                  nc.sync.dma_start(xgt_t[:], xgv[q:q + 128, r, :])
                  xgt = xgt_t[:]
                  gp = psw.tile([128, G4], F32, space="PSUM", tag="w")
                  nc.tensor.matmul(gp[:], lhsT=ht_t[:], rhs=whh_t[:],
                                   start=True, stop=True)
                  gsb = sb.tile([128, G4], F32, tag="gsb")
                  nc.vector.tensor_add(gsb[:], gp[:], xgt)
                  sg = sb.tile([128, 384], F32, tag="sg")
                  nc.scalar.activation(sg[:], gsb[:, 0:384], AF.Sigmoid)
                  tg = sb.tile([128, 128], F32, tag="tg")
                  nc.scalar.activation(tg[:], gsb[:, 384:512], AF.Tanh)
                  ig = sb.tile([128, 128], F32, tag="ig")
                  nc.vector.tensor_mul(ig[:], sg[:, 0:128], tg[:])
                  nc.vector.tensor_mul(c_t[:], c_t[:], sg[:, 128:256])
                  nc.vector.tensor_add(c_t[:], c_t[:], ig[:])
                  tc_ = sb.tile([128, 128], F32, tag="tc")
                  nc.scalar.activation(tc_[:], c_t[:], AF.Tanh)
                  if s >= K:
                      hout = h3_sb[:, (s - K) * 128:(s - K + 1) * 128]
                  else:
                      hs_ = sb.tile([128, 128], F32, tag="hs")
                      hout = hs_[:]
                  nc.vector.tensor_mul(hout, sg[:, 256:384], tc_[:])
                  if s < K + L - 1:
                      tp_ = ps.tile([128, 128], F32, space="PSUM", tag="tr")
                      nc.tensor.transpose(out=tp_[:], in_=hout, identity=ident_f[:])
                      nc.vector.tensor_copy(ht_t[:], tp_[:])
              if stop_after == "lstm":
                  raise _StopBuild

              # ------------ h3 -> scaled node rows -> AG3
              nc.sync.dma_start(
                  h3tmp.ap().rearrange("(l r) f -> l (r f)", r=L), h3_sb[:])
              h3n = ph.tile([128, L, D], F32, tag="big1")
              nc.sync.dma_start(h3n[:], h3tmp.ap().rearrange(
                  "(n p) d -> p n d", p=128))
              h3o = ph.tile([128, L, D], F32, tag="big2")
              for j in range(L):
                  nc.vector.tensor_scalar_mul(h3o[:, j, :], h3n[:, j, :],
                                              dc20_t[:, j:j + 1])
              nc.sync.dma_start(h3sc.ap().rearrange("(n p) d -> p n d", p=128),
                                h3o[:])
              nc.gpsimd.collective_compute(
                  "AllGather", mybir.AluOpType.bypass,
                  ins=[h3sc.ap()[0:SH, :].opt()],
                  outs=[table3.ap().opt()],
                  replica_groups=[list(range(NC))])
              if stop_after == "ag3":
                  raise _StopBuild

              # ------------ conv3 (own targets) -> z
              conv_pass(table3, acc3, gio_t, sio_t, tiles_o, NIDX_O)
              s3 = ph.tile([128, NT, 128], F32, tag="big1")
              load_reduce(acc3, s3)
              c3 = ph.tile([128, NT, 128], BF16, tag="cvt")
              nc.vector.tensor_copy(c3[:], s3[:])
              nc.sync.dma_start(s3d.ap().rearrange("(n p) d -> p n d", p=128),
                                c3[:])
              fm3 = ph.tile([128, NT, 128], BF16, tag="fm")
              nc.sync.dma_start(fm3[:], s3d.ap(), transpose=True)
              zsb = ph.tile([128, TP], BF16, tag="big2")
              for o in range(0, TP, 512):
                  p_ = psw.tile([128, 512], F32, space="PSUM", tag="w")
                  nc.tensor.matmul(p_[:], lhsT=wml_t[:],
                                   rhs=fm3[:].rearrange("p n d -> p (n d)")[:, o:o + 512],
                                   start=True, stop=True)
                  t_ = sb.tile([128, 512], F32, tag="zt")
                  nc.vector.tensor_mul(t_[:], p_[:], d1fm[:, K + o:K + o + 512])
                  nc.vector.tensor_scalar_add(zsb[:, o:o + 512], t_[:],
                                              bmlc_t[:, 0:1])
              nc.sync.dma_start(zmT.ap(), zsb[0:64, 0:SH])
              nc.sync.dma_start(zlT.ap(), zsb[64:128, 0:SH])
          except _StopBuild:
            pass
    nc.compile()
    return nc


class _StopBuild(Exception):
    pass


# ---------------------------------------------------------------- runner
_CACHE = {}


def _get_nc(pp, debug=False):
    key = (tuple(pp["tiles_o"]), tuple(pp["tiles_e"]))
    if key not in _CACHE:
        _CACHE[key] = build_nc(pp)
    return _CACHE[key]


def make_in_maps(inputs, pp):
    bf = ml_dtypes.bfloat16
    K = pp["K"]
    dinv = pp["dinv"]
    x = np.asarray(inputs["x"], np.float32)
    perm = np.concatenate([np.arange(0, 128), np.arange(128, 256),
                           np.arange(384, 512), np.arange(256, 384)])
    Wih = np.asarray(inputs["Wih"], np.float32)[perm]
    Whh = np.asarray(inputs["Whh"], np.float32)[perm]
    bias = (np.asarray(inputs["bih"], np.float32)
            + np.asarray(inputs["bhh"], np.float32))[perm]
    wml = np.concatenate([np.asarray(inputs["Wm"], np.float32),
                          np.asarray(inputs["Wl"], np.float32)], axis=1)
    bml = np.concatenate([np.asarray(inputs["bm"], np.float32),
                          np.asarray(inputs["bl"], np.float32)])

    base = {
        "w1": np.asarray(inputs["W1"], np.float32).astype(bf),
        "w2": np.asarray(inputs["W2"], np.float32).astype(bf),
        "wml": wml.astype(bf),
        "wiht": np.ascontiguousarray(Wih.T).astype(bf),
        "whht": np.ascontiguousarray(Whh.T).astype(np.float32),
        "biasg": bias[None, :].astype(bf),
        "b2c": np.asarray(inputs["b2"], np.float32)[:, None],
        "bmlc": bml[:, None],
        "b1r": np.asarray(inputs["b1"], np.float32)[None, :].astype(bf),
    }

    in_maps = []
    for c in range(NC):
        start = c * SH
        m = dict(base)
        xs = np.zeros((TP, D), np.float32)
        xs[0:SH] = x[start:start + SH]
        m["xt"] = np.ascontiguousarray(xs.T).astype(bf)
        # d1all: dinv over [start-K, start+TP) (0 outside [0, N))
        nodes = start - K + np.arange(K + TP)
        v = (nodes >= 0) & (nodes < N)
        da = np.zeros(K + TP, np.float32)
        da[v] = dinv[nodes[v]]
        m["d1all"] = da[None, :]
        # d1col/d2col: own-range per-tile columns [128, NT]
        down = np.zeros(TP, np.float32)
        nv = start + np.arange(TP)
        ov = nv < N
        down[ov] = dinv[nv[ov]]
        m["d1col"] = np.ascontiguousarray(down.reshape(NT, 128).T)
        m["d2col"] = np.ascontiguousarray((down * down).reshape(NT, 128).T)
        m["dcol20"] = np.ascontiguousarray(down.reshape(L, 128).T)
        mask = np.ones((128, NXB_HOST), np.float32)
        if c == 0:
            mask[:K, 0] = 0.0
        m["maskc"] = mask
        co = pp["cores"][c]
        m["gio"] = co["g_ou"]
        m["sio"] = co["s_ou"]
        m["gie"] = co["g_eu"]
        m["sie"] = co["s_eu"]
        in_maps.append(m)
    return in_maps


NXB_HOST = -(-(COVER + K_WARM) // 128)


def kernel(**inputs):
    pp = preprocess(np.asarray(inputs["edge_index"]))
    nc = _get_nc(pp)
    in_maps = make_in_maps(inputs, pp)
    res = run_bass_kernel_spmd(nc, in_maps, core_ids=list(range(NC)))
    zm = np.concatenate([res.results[c]["zmT"].astype(np.float32).T
                         for c in range(NC)], axis=0)
    zl = np.concatenate([res.results[c]["zlT"].astype(np.float32).T
                         for c in range(NC)], axis=0)
    return (np.ascontiguousarray(zm, dtype=np.float32),
            np.ascontiguousarray(zl, dtype=np.float32))


# revision 10
# speedup vs baseline: 1.0575x; 1.0575x over previous
"""Trainium2 Bass kernel for nn_Encoder_67190468378802 (GCN-LSTM encoder), v2.

Architecture (instruction-count-minimal for the axon backend, where every
instruction costs ~40-170us regardless of width):
 - GCN aggregation via dma_gather + dma_scatter_add in duplicate-free
   "rounds": each scatter instruction touches each accumulator slot at most
   once (R=8 replica slots per target cut round count to ~8; chunks of <=64
   tiles). Zero per-edge compute instructions.
 - W2/Wm/Wl commute past the aggregation (linear), applied in wide
   feature-major posts via a handful of 512-wide matmuls.
 - Phase 1 (X@W1) sharded across cores; 3 AllGathers share tables.
 - LSTM: truncated-window recurrence (K=32 warmup), 128 lanes x L=20,
   lane-major (unchanged from v1).
"""
import numpy as np
import ml_dtypes

import concourse.bacc as bacc
import concourse.bass as bass
import concourse.mybir as mybir
import concourse.tile as tile
from concourse.bass_utils import run_bass_kernel_spmd
from concourse.masks import make_identity

F32 = mybir.dt.float32
BF16 = mybir.dt.bfloat16
I16 = mybir.dt.int16
AF = mybir.ActivationFunctionType

N = 20000
NC = 8
SH = N // NC            # 2500
D = 128
G4 = 4 * D
LAT = 64
L = 20
LANES = 128
COVER = LANES * L       # 2560
K_WARM = 10
NT = 20                 # tiles per 2560-range
TP = NT * 128           # 2560 padded targets (own and ext both fit)
REP = 8                 # replica slots per target
TRASH = REP * TP        # trash slot base (rows TRASH..TRASH+127)
ACC_ROWS = REP * TP + 128
CHUNK_TILES = 40        # max gather/scatter tiles per instruction


# ---------------------------------------------------------------- host prep
def _pack_rounds(src, tloc):
    """Duplicate-free round/replica packing.

    Returns (gidx, sidx, chunks): int16 arrays of equal padded length
    (multiple of 128), and a list of per-chunk tile counts. Within any chunk
    all scatter slots are distinct (except the trash slot).
    """
    order = np.lexsort((np.arange(len(tloc)), tloc))
    t_s = tloc[order]
    s_s = src[order]
    # rank of each edge within its target
    uniq, starts = np.unique(t_s, return_index=True)
    rank = np.arange(len(t_s)) - np.repeat(starts, np.diff(np.append(starts, len(t_s))))
    rep = rank % REP
    rnd = rank // REP
    slot = rep * TP + t_s
    nrounds = int(rnd.max()) + 1 if len(rnd) else 0

    gl, sl, chunks = [], [], []
    for r in range(nrounds):
        m = rnd == r
        g = s_s[m]
        s = slot[m]
        n = len(g)
        npad = -(-n // 128) * 128
        gpad = np.zeros(npad, np.int64)
        spad = np.full(npad, TRASH, np.int64)
        gpad[:n] = g
        spad[:n] = s
        ntiles = npad // 128
        o = 0
        while o < ntiles:
            c = min(CHUNK_TILES, ntiles - o)
            chunks.append(c)
            gl.append(gpad[o * 128:(o + c) * 128])
            sl.append(spad[o * 128:(o + c) * 128])
            o += c
    gidx = np.concatenate(gl).astype(np.int16)
    sidx = np.concatenate(sl).astype(np.int16)
    return gidx, sidx, chunks


def _wrap16(idx):
    return np.ascontiguousarray(idx.reshape(-1, 16).T)


def preprocess(edge_index):
    K = K_WARM
    row = np.asarray(edge_index[0], dtype=np.int64)
    col = np.asarray(edge_index[1], dtype=np.int64)
    loop = np.arange(N, dtype=np.int64)
    row = np.concatenate([row, loop])
    col = np.concatenate([col, loop])
    deg = np.bincount(col, minlength=N).astype(np.float64)
    dinv = (1.0 / np.sqrt(deg)).astype(np.float32)

    cores = []
    for c in range(NC):
        start = c * SH
        # own set: targets in [start, start+SH)
        mo = (col >= start) & (col < start + SH)
        g_o, s_o, ch_o = _pack_rounds(row[mo], col[mo] - start)
        # ext set: targets in [start-K, start+SH)
        me = (col >= start - K) & (col < start + SH)
        g_e, s_e, ch_e = _pack_rounds(row[me], col[me] - (start - K))
        cores.append(dict(g_o=g_o, s_o=s_o, ch_o=ch_o,
                          g_e=g_e, s_e=s_e, ch_e=ch_e))

    # shared shapes: pad all cores' index arrays to the max length & chunk
    # schedule must be identical across cores (SPMD single program) -> pad
    # chunk lists with empty?? simplest: use the max schedule; shorter cores
    # pad with full-trash chunks.
    def unify(key_g, key_s, key_ch):
        nch = max(len(co[key_ch]) for co in cores)
        tiles = [max(co[key_ch][i] if i < len(co[key_ch]) else 1 for co in cores)
                 for i in range(nch)]
        for co in cores:
            gl, sl = [], []
            off = 0
            for i, t in enumerate(tiles):
                have = co[key_ch][i] if i < len(co[key_ch]) else 0
                g = np.zeros(t * 128, np.int64)
                s = np.full(t * 128, TRASH, np.int64)
                if have:
                    g[:have * 128] = co[key_g][off:off + have * 128]
                    s[:have * 128] = co[key_s][off:off + have * 128]
                    off += have * 128
                gl.append(g)
                sl.append(s)
            co[key_g + "u"] = _wrap16(np.concatenate(gl).astype(np.int16))
            co[key_s + "u"] = _wrap16(np.concatenate(sl).astype(np.int16))
        return tiles

    tiles_o = unify("g_o", "s_o", "ch_o")
    tiles_e = unify("g_e", "s_e", "ch_e")
    return dict(dinv=dinv, cores=cores, tiles_o=tiles_o, tiles_e=tiles_e, K=K)


# ---------------------------------------------------------------- device
def build_nc(pp, reps=1, stop_after=None):
    K = pp["K"]
    tiles_o, tiles_e = pp["tiles_o"], pp["tiles_e"]
    NIDX_O = sum(tiles_o) * 128
    NIDX_E = sum(tiles_e) * 128
    NXB = -(-(COVER + K) // 128)       # 21 xg row blocks
    NTH = max(NT, NXB)                 # 21
    XGR = NXB * 128
    XGROWS = -(-XGR // L) * L + L * 8

    nc = bacc.Bacc(None, target_bir_lowering=False)

    # ---------------- inputs
    xt = nc.dram_tensor("xt", [D, TP], BF16, kind="ExternalInput")
    w1 = nc.dram_tensor("w1", [D, D], BF16, kind="ExternalInput")
    w2 = nc.dram_tensor("w2", [D, D], BF16, kind="ExternalInput")
    wml = nc.dram_tensor("wml", [D, D], BF16, kind="ExternalInput")
    wiht = nc.dram_tensor("wiht", [D, G4], BF16, kind="ExternalInput")
    whht = nc.dram_tensor("whht", [D, G4], F32, kind="ExternalInput")
    biasg = nc.dram_tensor("biasg", [1, G4], BF16, kind="ExternalInput")
    b2c = nc.dram_tensor("b2c", [D, 1], F32, kind="ExternalInput")
    bmlc = nc.dram_tensor("bmlc", [D, 1], F32, kind="ExternalInput")
    b1r = nc.dram_tensor("b1r", [1, D], BF16, kind="ExternalInput")
    d1all = nc.dram_tensor("d1all", [1, K + TP], F32, kind="ExternalInput")
    d1col = nc.dram_tensor("d1col", [128, NT], F32, kind="ExternalInput")
    d2col = nc.dram_tensor("d2col", [128, NT], F32, kind="ExternalInput")
    dcol20 = nc.dram_tensor("dcol20", [128, L], F32, kind="ExternalInput")
    maskc = nc.dram_tensor("maskc", [128, NXB], F32, kind="ExternalInput")
    gio = nc.dram_tensor("gio", [16, NIDX_O // 16], I16, kind="ExternalInput")
    sio = nc.dram_tensor("sio", [16, NIDX_O // 16], I16, kind="ExternalInput")
    gie = nc.dram_tensor("gie", [16, NIDX_E // 16], I16, kind="ExternalInput")
    sie = nc.dram_tensor("sie", [16, NIDX_E // 16], I16, kind="ExternalInput")
    gior = nc.dram_tensor("gior", [128, NIDX_O // 16], I16)
    sior = nc.dram_tensor("sior", [128, NIDX_O // 16], I16)
    gier = nc.dram_tensor("gier", [128, NIDX_E // 16], I16)
    sier = nc.dram_tensor("sier", [128, NIDX_E // 16], I16)

    # ---------------- outputs
    zmT = nc.dram_tensor("zmT", [LAT, SH], BF16, kind="ExternalOutput")
    zlT = nc.dram_tensor("zlT", [LAT, SH], BF16, kind="ExternalOutput")

    # ---------------- internal DRAM
    t1loc = nc.dram_tensor("t1loc", [TP, D], F32)
    table1 = nc.dram_tensor("table1", [N, D], F32, addr_space="Shared")
    t2loc = nc.dram_tensor("t2loc", [TP, D], F32)
    table2 = nc.dram_tensor("table2", [N, D], F32, addr_space="Shared")
    acc1 = nc.dram_tensor("acc1", [ACC_ROWS, D], F32)
    acc2 = nc.dram_tensor("acc2", [ACC_ROWS, D], F32)
    acc3 = nc.dram_tensor("acc3", [ACC_ROWS, D], F32)
    s2d = nc.dram_tensor("s2d", [TP, D], BF16)
    s3d = nc.dram_tensor("s3d", [TP, D], BF16)
    xg_dram = nc.dram_tensor("xg_dram", [XGROWS, G4], BF16)
    h3tmp = nc.dram_tensor("h3tmp", [COVER, D], F32)
    h3sc = nc.dram_tensor("h3sc", [COVER, D], F32)
    table3 = nc.dram_tensor("table3", [N, D], F32, addr_space="Shared")

    with tile.TileContext(nc) as tc:
        import contextlib
        ctx = contextlib.ExitStack()
        with ctx:
          try:
            const = ctx.enter_context(tc.tile_pool(name="const", bufs=1))
            sb = ctx.enter_context(tc.tile_pool(name="sb", bufs=2))
            ph = ctx.enter_context(tc.tile_pool(name="ph", bufs=1))
            ar = ctx.enter_context(tc.tile_pool(name="ar", bufs=2))
            gat = ctx.enter_context(tc.tile_pool(name="gat", bufs=2))
            ps = ctx.enter_context(tc.tile_pool(name="ps", bufs=2, space="PSUM"))
            psw = ctx.enter_context(tc.tile_pool(name="psw", bufs=2, space="PSUM"))

            # ------------ constants
            gio_t = const.tile([128, NIDX_O // 16], I16)
            sio_t = const.tile([128, NIDX_O // 16], I16)
            gie_t = const.tile([128, NIDX_E // 16], I16)
            sie_t = const.tile([128, NIDX_E // 16], I16)
            for tdst, tsrc, trep in ((gio_t, gio, gior), (sio_t, sio, sior),
                                     (gie_t, gie, gier), (sie_t, sie, sier)):
                for k in range(8):
                    nc.sync.dma_start(trep.ap()[16 * k:16 * (k + 1), :],
                                      tsrc.ap())
                nc.sync.dma_start(tdst[:], trep.ap())
            w1_t = const.tile([128, D], BF16)
            nc.sync.dma_start(w1_t[:], w1[:])
            w2_t = const.tile([128, D], BF16)
            nc.sync.dma_start(w2_t[:], w2[:])
            wml_t = const.tile([128, D], BF16)
            nc.sync.dma_start(wml_t[:], wml[:])
            wih_t = const.tile([128, G4], BF16)
            nc.sync.dma_start(wih_t[:], wiht[:])
            whh_t = const.tile([128, G4], F32)
            nc.sync.dma_start(whh_t[:], whht[:])
            biasg_t = const.tile([1, G4], BF16)
            nc.sync.dma_start(biasg_t[:], biasg[:])
            b2c_t = const.tile([128, 1], F32)
            nc.sync.dma_start(b2c_t[:], b2c[:])
            bmlc_t = const.tile([128, 1], F32)
            nc.sync.dma_start(bmlc_t[:], bmlc[:])
            b1r_t = const.tile([1, D], BF16)
            nc.sync.dma_start(b1r_t[:], b1r[:])
            d1all_t = const.tile([1, K + TP], F32)
            nc.sync.dma_start(d1all_t[:], d1all[:])
            d1col_t = const.tile([128, NT], F32)
            nc.sync.dma_start(d1col_t[:], d1col[:])
            d2col_t = const.tile([128, NT], F32)
            nc.sync.dma_start(d2col_t[:], d2col[:])
            dc20_t = const.tile([128, L], F32)
            nc.sync.dma_start(dc20_t[:], dcol20[:])
            mask_t = const.tile([128, NXB], F32)
            nc.sync.dma_start(mask_t[:], maskc[:])
            ones1_f = const.tile([1, 128], F32)
            nc.vector.memset(ones1_f[:], 1.0)
            ones1_bf = const.tile([1, 128], BF16)
            nc.vector.memset(ones1_bf[:], 1.0)
            ident_f = const.tile([128, 128], F32)
            make_identity(nc, ident_f[:])
            ident_bf = const.tile([128, 128], BF16)
            make_identity(nc, ident_bf[:])
            zeros_t = const.tile([128, 24, 128], F32)
            nc.vector.memset(zeros_t[:], 0.0)

            # d1fm_all [128, K+TP]: dinv over [start-K, start+TP), bcast over
            # partitions (feature-major free-axis scale)
            d1fm = const.tile([128, K + TP], BF16)
            o = 0
            while o < K + TP:
                w_ = min(512, K + TP - o)
                p_ = psw.tile([128, 512], F32, space="PSUM", tag="w")
                nc.tensor.matmul(p_[:, :w_], lhsT=ones1_f[:],
                                 rhs=d1all_t[:, o:o + w_], start=True, stop=True)
                nc.vector.tensor_copy(d1fm[:, o:o + w_], p_[:, :w_])
                o += w_

            # b1w [128, 128] = b1 broadcast over partitions; then
            # b1d [128, NT, 128] = d1(own node) * b1(d)
            b1w_p = psw.tile([128, 512], F32, space="PSUM", tag="w")
            nc.tensor.matmul(b1w_p[:, 0:128], lhsT=ones1_bf[:], rhs=b1r_t[:],
                             start=True, stop=True)
            b1w = const.tile([128, 128], F32)
            nc.vector.tensor_copy(b1w[:], b1w_p[:, 0:128])
            b1d = const.tile([128, NT, 128], BF16)
            for n in range(NT):
                nc.vector.tensor_scalar_mul(b1d[:, n, :], b1w[:],
                                            d1col_t[:, n:n + 1])

            # biasw [128, G4] = biasg broadcast over partitions
            biasw = const.tile([128, G4], F32)
            for o in range(0, G4, 512):
                bp = psw.tile([128, 512], F32, space="PSUM", tag="w")
                nc.tensor.matmul(bp[:], lhsT=ones1_bf[:],
                                 rhs=biasg_t[:, o:o + 512], start=True, stop=True)
                nc.vector.tensor_copy(biasw[:, o:o + 512], bp[:])

            # h2t feature-major [128, NTH*128]; tail zero
            h2t = const.tile([128, NTH * 128], BF16)
            nc.vector.memset(h2t[:, NT * 128:], 0.0)
            h3_sb = const.tile([128, COVER], F32)

            # zero accumulators
            for acc in (acc1, acc2, acc3):
                o = 0
                v = acc.ap().rearrange("(a p) d -> p a d", p=128)
                a_total = ACC_ROWS // 128
                while o < a_total:
                    w_ = min(24, a_total - o)
                    nc.sync.dma_start(v[:, o:o + w_, :], zeros_t[:, 0:w_, :])
                    o += w_

            for _rep in range(reps):
              # ------------ phase 1: t1loc = dinv * (X @ W1) (own shard)
              xt_sb = ph.tile([128, TP], BF16, tag="big3")
              nc.sync.dma_start(xt_sb[:], xt.ap())
              t1fm = ph.tile([128, TP], BF16, tag="cvt")
              for o in range(0, TP, 512):
                  p_ = psw.tile([128, 512], F32, space="PSUM", tag="w")
                  nc.tensor.matmul(p_[:], lhsT=w1_t[:], rhs=xt_sb[:, o:o + 512],
                                   start=True, stop=True)
                  nc.vector.tensor_mul(t1fm[:, o:o + 512], p_[:],
                                       d1fm[:, K + o:K + o + 512])
              nc.sync.dma_start(s2d.ap().rearrange("(n p) d -> p n d", p=128),
                                t1fm[:].rearrange("p (n d) -> p n d", d=128))
              t1nm = ph.tile([128, NT, 128], BF16, tag="fm")
              nc.sync.dma_start(t1nm[:], s2d.ap(), transpose=True)
              t1sb = ph.tile([128, NT, 128], F32, tag="big2")
              nc.vector.tensor_copy(t1sb[:], t1nm[:])
              nc.sync.dma_start(t1loc.ap().rearrange("(n p) d -> p n d", p=128),
                                t1sb[:])
              nc.gpsimd.collective_compute(
                  "AllGather", mybir.AluOpType.bypass,
                  ins=[t1loc.ap()[0:SH, :].opt()],
                  outs=[table1.ap().opt()],
                  replica_groups=[list(range(NC))])
              if stop_after == "p1":
                  raise _StopBuild

              # ------------ conv pass helper: gather+convert+scatter rounds
              def conv_pass(table, acc, gi_t, si_t, tiles_sched, nidx):
                  off = 0
                  for ctiles in tiles_sched:
                      gsz = ctiles * 128
                      gt = gat.tile([128, CHUNK_TILES, D], F32, tag="g")
                      nc.gpsimd.dma_gather(
                          gt[:, 0:ctiles, :], table.ap()[:],
                          gi_t[:, off // 16:(off + gsz) // 16],
                          gsz, gsz, D, single_packet=False)
                      nc.gpsimd.dma_scatter_add(
                          acc.ap()[:], gt[:, 0:ctiles, :],
                          si_t[:, off // 16:(off + gsz) // 16],
                          gsz, gsz, D, single_packet=False)
                      off += gsz

              def load_reduce(acc, out):
                  """accumulating DMA loads: out = sum_r acc[rep r] (CCE add)"""
                  v = acc.ap()[0:REP * TP, :].rearrange(
                      "(r n p) d -> p r n d", p=128, r=REP)
                  nc.sync.dma_start(out[:], v[:, 0, :, :])
                  for r in range(1, REP):
                      nc.gpsimd.dma_start(out[:], v[:, r, :, :],
                                          accum_op=mybir.AluOpType.add)

              # ------------ conv1 -> table2 rows (node-major post)
              conv_pass(table1, acc1, gio_t, sio_t, tiles_o, NIDX_O)
              s1 = ph.tile([128, NT, 128], F32, tag="big1")
              load_reduce(acc1, s1)
              for n in range(NT):
                  nc.vector.tensor_scalar_mul(s1[:, n, :], s1[:, n, :],
                                              d2col_t[:, n:n + 1])
              nc.vector.tensor_add(s1[:], s1[:], b1d[:])
              o1 = ph.tile([128, NT, 128], F32, tag="big2")
              nc.scalar.activation(o1[:], s1[:], AF.Relu)
              nc.sync.dma_start(t2loc.ap().rearrange("(n p) d -> p n d", p=128),
                                o1[:])
              nc.gpsimd.collective_compute(
                  "AllGather", mybir.AluOpType.bypass,
                  ins=[t2loc.ap()[0:SH, :].opt()],
                  outs=[table2.ap().opt()],
                  replica_groups=[list(range(NC))])
              if stop_after == "conv1":
                  raise _StopBuild

              # ------------ conv2 -> h2t feature-major (ext targets)
              conv_pass(table2, acc2, gie_t, sie_t, tiles_e, NIDX_E)
              s2 = ph.tile([128, NT, 128], F32, tag="big1")
              load_reduce(acc2, s2)
              c2 = ph.tile([128, NT, 128], BF16, tag="cvt")
              nc.vector.tensor_copy(c2[:], s2[:])
              nc.sync.dma_start(s2d.ap().rearrange("(n p) d -> p n d", p=128),
                                c2[:])
              fm2 = ph.tile([128, NT, 128], BF16, tag="fm")
              nc.sync.dma_start(fm2[:], s2d.ap(), transpose=True)
              for o in range(0, TP, 512):
                  p_ = psw.tile([128, 512], F32, space="PSUM", tag="w")
                  nc.tensor.matmul(p_[:], lhsT=w2_t[:],
                                   rhs=fm2[:].rearrange("p n d -> p (n d)")[:, o:o + 512],
                                   start=True, stop=True)
                  t_ = sb.tile([128, 512], F32, tag="t2f")
                  nc.vector.tensor_mul(t_[:], p_[:], d1fm[:, o:o + 512])
                  nc.scalar.activation(h2t[:, o:o + 512], t_[:], AF.Relu,
                                       bias=b2c_t[:, 0:1])
              if stop_after == "conv2":
                  raise _StopBuild

              # ------------ xg = H2T.T @ WihT + bias (masked)
              for b in range(NXB):
                  p_ = psw.tile([128, G4], F32, space="PSUM", tag="w")
                  nc.tensor.matmul(p_[:], lhsT=h2t[:, b * 128:(b + 1) * 128],
                                   rhs=wih_t[:], start=True, stop=True)
                  ob = sb.tile([128, G4], F32, tag="xgb")
                  nc.vector.tensor_add(ob[:], p_[:], biasw[:])
                  o_ = sb.tile([128, G4], BF16, tag="xgo")
                  nc.vector.tensor_scalar_mul(o_[:], ob[:], mask_t[:, b:b + 1])
                  nc.sync.dma_start(xg_dram.ap()[b * 128:(b + 1) * 128, :], o_[:])
              if stop_after == "xg":
                  raise _StopBuild

              # ------------ LSTM (lane-major, K warmup)
              c_t = const.tile([128, D], F32)
              nc.vector.memset(c_t[:], 0.0)
              ht_t = const.tile([128, D], F32)
              nc.vector.memset(ht_t[:], 0.0)
              xgv = xg_dram.ap().rearrange("(l r) g -> l r g", r=L)
              for s in range(K + L):
                  q, r = divmod(s, L)
                  xgt_t = sb.tile([128, G4], BF16, tag="xgl")
                  nc.sync.dma_start(xgt_t[:], xgv[q:q + 128, r, :])
                  xgt = xgt_t[:]
                  gp = psw.tile([128, G4], F32, space="PSUM", tag="w")
                  nc.tensor.matmul(gp[:], lhsT=ht_t[:], rhs=whh_t[:],
                                   start=True, stop=True)
                  gsb = sb.tile([128, G4], F32, tag="gsb")
                  nc.vector.tensor_add(gsb[:], gp[:], xgt)
                  sg = sb.tile([128, 384], F32, tag="sg")
                  nc.scalar.activation(sg[:], gsb[:, 0:384], AF.Sigmoid)
                  tg = sb.tile([128, 128], F32, tag="tg")
                  nc.scalar.activation(tg[:], gsb[:, 384:512], AF.Tanh)
                  ig = sb.tile([128, 128], F32, tag="ig")
                  nc.vector.tensor_mul(ig[:], sg[:, 0:128], tg[:])
                  nc.vector.tensor_mul(c_t[:], c_t[:], sg[:, 128:256])
                  nc.vector.tensor_add(c_t[:], c_t[:], ig[:])
                  tc_ = sb.tile([128, 128], F32, tag="tc")
                  nc.scalar.activation(tc_[:], c_t[:], AF.Tanh)
                  if s >= K:
                      hout = h3_sb[:, (s - K) * 128:(s - K + 1) * 128]
                  else:
                      hs_ = sb.tile([128, 128], F32, tag="hs")
                      hout = hs_[:]
                  nc.vector.tensor_mul(hout, sg[:, 256:384], tc_[:])
                  if s < K + L - 1:
                      tp_ = ps.tile([128, 128], F32, space="PSUM", tag="tr")
                      nc.tensor.transpose(out=tp_[:], in_=hout, identity=ident_f[:])
                      nc.vector.tensor_copy(ht_t[:], tp_[:])
              if stop_after == "lstm":
                  raise _StopBuild

              # ------------ h3 -> scaled node rows -> AG3
              nc.sync.dma_start(
                  h3tmp.ap().rearrange("(l r) f -> l (r f)", r=L), h3_sb[:])
              h3n = ph.tile([128, L, D], F32, tag="big1")
              nc.sync.dma_start(h3n[:], h3tmp.ap().rearrange(
                  "(n p) d -> p n d", p=128))
              h3o = ph.tile([128, L, D], F32, tag="big2")
              for j in range(L):
                  nc.vector.tensor_scalar_mul(h3o[:, j, :], h3n[:, j, :],
                                              dc20_t[:, j:j + 1])
              nc.sync.dma_start(h3sc.ap().rearrange("(n p) d -> p n d", p=128),
                                h3o[:])
              nc.gpsimd.collective_compute(
                  "AllGather", mybir.AluOpType.bypass,
                  ins=[h3sc.ap()[0:SH, :].opt()],
                  outs=[table3.ap().opt()],
                  replica_groups=[list(range(NC))])
              if stop_after == "ag3":
                  raise _StopBuild

              # ------------ conv3 (own targets) -> z
              conv_pass(table3, acc3, gio_t, sio_t, tiles_o, NIDX_O)
              s3 = ph.tile([128, NT, 128], F32, tag="big1")
              load_reduce(acc3, s3)
              c3 = ph.tile([128, NT, 128], BF16, tag="cvt")
              nc.vector.tensor_copy(c3[:], s3[:])
              nc.sync.dma_start(s3d.ap().rearrange("(n p) d -> p n d", p=128),
                                c3[:])
              fm3 = ph.tile([128, NT, 128], BF16, tag="fm")
              nc.sync.dma_start(fm3[:], s3d.ap(), transpose=True)
              zsb = ph.tile([128, TP], BF16, tag="big2")
              for o in range(0, TP, 512):
                  p_ = psw.tile([128, 512], F32, space="PSUM", tag="w")
                  nc.tensor.matmul(p_[:], lhsT=wml_t[:],
                                   rhs=fm3[:].rearrange("p n d -> p (n d)")[:, o:o + 512],
                                   start=True, stop=True)
                  t_ = sb.tile([128, 512], F32, tag="zt")
                  nc.vector.tensor_mul(t_[:], p_[:], d1fm[:, K + o:K + o + 512])
                  nc.vector.tensor_scalar_add(zsb[:, o:o + 512], t_[:],
                                              bmlc_t[:, 0:1])
              nc.sync.dma_start(zmT.ap(), zsb[0:64, 0:SH])
              nc.sync.dma_start(zlT.ap(), zsb[64:128, 0:SH])
          except _StopBuild:
            pass
    nc.compile()
    return nc


class _StopBuild(Exception):
    pass


# ---------------------------------------------------------------- runner
_CACHE = {}


def _get_nc(pp, debug=False):
    key = (tuple(pp["tiles_o"]), tuple(pp["tiles_e"]))
    if key not in _CACHE:
        _CACHE[key] = build_nc(pp)
    return _CACHE[key]


def make_in_maps(inputs, pp):
    bf = ml_dtypes.bfloat16
    K = pp["K"]
    dinv = pp["dinv"]
    x = np.asarray(inputs["x"], np.float32)
    perm = np.concatenate([np.arange(0, 128), np.arange(128, 256),
                           np.arange(384, 512), np.arange(256, 384)])
    Wih = np.asarray(inputs["Wih"], np.float32)[perm]
    Whh = np.asarray(inputs["Whh"], np.float32)[perm]
    bias = (np.asarray(inputs["bih"], np.float32)
            + np.asarray(inputs["bhh"], np.float32))[perm]
    wml = np.concatenate([np.asarray(inputs["Wm"], np.float32),
                          np.asarray(inputs["Wl"], np.float32)], axis=1)
    bml = np.concatenate([np.asarray(inputs["bm"], np.float32),
                          np.asarray(inputs["bl"], np.float32)])

    base = {
        "w1": np.asarray(inputs["W1"], np.float32).astype(bf),
        "w2": np.asarray(inputs["W2"], np.float32).astype(bf),
        "wml": wml.astype(bf),
        "wiht": np.ascontiguousarray(Wih.T).astype(bf),
        "whht": np.ascontiguousarray(Whh.T).astype(np.float32),
        "biasg": bias[None, :].astype(bf),
        "b2c": np.asarray(inputs["b2"], np.float32)[:, None],
        "bmlc": bml[:, None],
        "b1r": np.asarray(inputs["b1"], np.float32)[None, :].astype(bf),
    }

    in_maps = []
    for c in range(NC):
        start = c * SH
        m = dict(base)
        xs = np.zeros((TP, D), np.float32)
        xs[0:SH] = x[start:start + SH]
        m["xt"] = np.ascontiguousarray(xs.T).astype(bf)
        # d1all: dinv over [start-K, start+TP) (0 outside [0, N))
        nodes = start - K + np.arange(K + TP)
        v = (nodes >= 0) & (nodes < N)
        da = np.zeros(K + TP, np.float32)
        da[v] = dinv[nodes[v]]
        m["d1all"] = da[None, :]
        # d1col/d2col: own-range per-tile columns [128, NT]
        down = np.zeros(TP, np.float32)
        nv = start + np.arange(TP)
        ov = nv < N
        down[ov] = dinv[nv[ov]]
        m["d1col"] = np.ascontiguousarray(down.reshape(NT, 128).T)
        m["d2col"] = np.ascontiguousarray((down * down).reshape(NT, 128).T)
        m["dcol20"] = np.ascontiguousarray(down.reshape(L, 128).T)
        mask = np.ones((128, NXB_HOST), np.float32)
        if c == 0:
            mask[:K, 0] = 0.0
        m["maskc"] = mask
        co = pp["cores"][c]
        m["gio"] = co["g_ou"]
        m["sio"] = co["s_ou"]
        m["gie"] = co["g_eu"]
        m["sie"] = co["s_eu"]
        in_maps.append(m)
    return in_maps


NXB_HOST = -(-(COVER + K_WARM) // 128)


def kernel(**inputs):
    import time as _time
    pp = preprocess(np.asarray(inputs["edge_index"]))
    nc = _get_nc(pp)
    in_maps = make_in_maps(inputs, pp)
    res = None
    for attempt in range(3):
        try:
            res = run_bass_kernel_spmd(nc, in_maps, core_ids=list(range(NC)))
            break
        except Exception:
            # transient device wedge (NRT_EXEC_UNIT_UNRECOVERABLE) -- back
            # off and retry; re-raise on final attempt
            if attempt == 2:
                raise
            _time.sleep(5)
    zm = np.concatenate([res.results[c]["zmT"].astype(np.float32).T
                         for c in range(NC)], axis=0)
    zl = np.concatenate([res.results[c]["zlT"].astype(np.float32).T
                         for c in range(NC)], axis=0)
    return (np.ascontiguousarray(zm, dtype=np.float32),
            np.ascontiguousarray(zl, dtype=np.float32))
